# revision 5
# baseline (speedup 1.0000x reference)
"""Trainium2 kernel for nn_KernelizedAttention_14869176779022.

Math note: the reference computes
    out = (s * v) / s        with s = <phi_q, phi_k> > 0  (sums of exps)
so out == v == x @ Wv.T + bv exactly (up to one multiply/divide rounding).
The kernel therefore only computes the Wv linear layer.

Sharding: data-parallel over the 8192 (B*S) positions — 1024 rows per core.
Wv (pre-transposed on host) and bv are replicated. x rows are pre-swizzled on
the host into the exact SBUF layout the TensorEngine wants (contraction dim on
partitions), so every DMA is a contiguous 4KB-per-partition read.
"""

import sys

if "/opt/trn_rl_repo" not in sys.path:
    sys.path.insert(0, "/opt/trn_rl_repo")

import numpy as np

B, S, E = 2, 4096, 1024
N_CORES = 8
ROWS = B * S            # 8192
R = ROWS // N_CORES     # 1024 rows per core
P = 128                 # partitions
KT = E // P             # 8 contraction tiles
MT = R // P             # 8 row tiles per core
NSZ = 512               # PSUM bank free size (fp32)
NT = E // NSZ           # 2 output-column tiles

_NC_CACHE = {}


def _build_nc(**bass_kwargs):
    import concourse.bass as bass
    import concourse.mybir as mybir
    from concourse import bacc
    from concourse.tile import TileContext

    f32 = mybir.dt.float32
    nc = bacc.Bacc(None, target_bir_lowering=False, **bass_kwargs)

    # xb[m, p, kt*P + mm] = x_shard[m*P + mm, kt*P + p]  (host pre-swizzled)
    xb = nc.dram_tensor("xb", [MT, P, KT * P], f32, kind="ExternalInput")
    # wvT[k, j] = Wv[j, k]
    wvT = nc.dram_tensor("wvT", [E, E], f32, kind="ExternalInput")
    bv = nc.dram_tensor("bv", [1, E], f32, kind="ExternalInput")
    out = nc.dram_tensor("out", [R, E], f32, kind="ExternalOutput")

    with TileContext(nc) as tc:
        with (
            tc.tile_pool(name="consts", bufs=1) as consts,
            tc.tile_pool(name="xpool", bufs=3) as xpool,
            tc.tile_pool(name="opool", bufs=3) as opool,
            tc.tile_pool(name="ppool", bufs=4, space="PSUM") as ppool,
        ):
            # bias broadcast to all 128 partitions
            bias_sb = consts.tile([P, E], f32, tag="bias")
            bias_bcast = bass.AP(
                tensor=bv.tensor if hasattr(bv, "tensor") else bv,
                offset=0,
                ap=[[0, P], [1, E]],
            )
            nc.gpsimd.dma_start(out=bias_sb, in_=bias_bcast)

            # whole Wv^T in SBUF: wv_sb[p, kt, j] = wvT[kt*P + p, j]
            wv_sb = consts.tile([P, KT, E], f32, tag="wv")
            nc.sync.dma_start(
                out=wv_sb, in_=wvT.rearrange("(kt p) j -> p kt j", p=P)
            )

            for m in range(MT):
                xm = xpool.tile([P, KT * P], f32, tag="xm")
                nc.sync.dma_start(out=xm, in_=xb[m])
                om = opool.tile([P, E], f32, tag="om")
                for n in range(NT):
                    ps = ppool.tile([P, NSZ], f32, tag="ps")
                    for k in range(KT):
                        nc.tensor.matmul(
                            ps,
                            xm[:, k * P : (k + 1) * P],
                            wv_sb[:, k, n * NSZ : (n + 1) * NSZ],
                            start=(k == 0),
                            stop=(k == KT - 1),
                        )
                    nc.vector.tensor_add(
                        out=om[:, n * NSZ : (n + 1) * NSZ],
                        in0=ps,
                        in1=bias_sb[:, n * NSZ : (n + 1) * NSZ],
                    )
                nc.sync.dma_start(out=out[m * P : (m + 1) * P, :], in_=om)
    nc.compile()
    return nc


def _get_nc():
    if "nc" not in _NC_CACHE:
        _NC_CACHE["nc"] = _build_nc()
    return _NC_CACHE["nc"]


def _prep_in_maps(x, Wv, bv):
    x = np.ascontiguousarray(np.asarray(x, dtype=np.float32))
    Wv = np.asarray(Wv, dtype=np.float32)
    bv = np.asarray(bv, dtype=np.float32)

    xf = x.reshape(ROWS, E)
    wvT = np.ascontiguousarray(Wv.T)
    bv2 = np.ascontiguousarray(bv.reshape(1, E))

    in_maps = []
    for c in range(N_CORES):
        xs = xf[c * R : (c + 1) * R]                    # [R, E]
        # xb[m, p, kt*P+mm] = xs[m*P+mm, kt*P+p]
        xbc = np.ascontiguousarray(
            xs.reshape(MT, P, KT, P).transpose(0, 3, 2, 1).reshape(MT, P, KT * P)
        )
        in_maps.append({"xb": xbc, "wvT": wvT, "bv": bv2})
    return in_maps


def _install_ntff_hook():
    """This image's antenv lacks axon_hooks; recreate the bridge module so
    run_bass_kernel_spmd(trace=True) can reach the ctypes NTFF profiler."""
    import types

    if "antenv.axon_hooks" in sys.modules:
        return
    try:
        from trn_agent_boot.trn_boot import _ntff_profile_via_ctypes
    except ImportError:
        return
    hook = _ntff_profile_via_ctypes("/opt/axon/libaxon_pjrt.so")
    mod = types.ModuleType("antenv.axon_hooks")
    mod._hook = hook
    mod.get_axon_ntff_profile_hook = lambda: mod._hook
    mod.set_axon_ntff_profile_hook = lambda h: setattr(mod, "_hook", h)
    sys.modules["antenv.axon_hooks"] = mod


def _run(x, Wv, bv, trace=False):
    from concourse.bass_utils import run_bass_kernel_spmd

    if trace:
        _install_ntff_hook()
    nc = _get_nc()
    in_maps = _prep_in_maps(x, Wv, bv)
    res = run_bass_kernel_spmd(
        nc, in_maps, core_ids=list(range(N_CORES)), trace=trace
    )
    out = np.concatenate([res.results[c]["out"] for c in range(N_CORES)], axis=0)
    return out.reshape(B, S, E).astype(np.float32), res


def kernel(x, Wq, bq, Wk, bk, Wv, bv, weights):
    out, _ = _run(x, Wv, bv, trace=False)
    return out


def kernel_traced(x, Wq, bq, Wk, bk, Wv, bv, weights):
    """Like kernel() but with NTFF profiling; returns (out, BassKernelResults)."""
    out, res = _run(x, Wv, bv, trace=True)
    return out, res


# revision 7
# speedup vs baseline: 2.3652x; 2.3652x over previous
"""Trainium2 kernel for nn_KernelizedAttention_14869176779022.

Math note: the reference computes
    out = (s * v) / s        with s = <phi_q, phi_k> > 0  (sums of exps)
so out == v == x @ Wv.T + bv exactly (up to one multiply/divide rounding).
The kernel therefore only computes the Wv linear layer.

Sharding: data-parallel over the 8192 (B*S) positions — 1024 rows per core.
Wv (pre-transposed) and bv are replicated. x is pre-swizzled on the host into
the exact SBUF layout the TensorEngine wants (contraction dim on partitions),
so every DMA is a few large per-partition-contiguous transfers via SWDGE.

Compute dtype bf16 (fp32 PSUM accumulation): fp32 matmul on TRN2 runs as a
HI/LO double pass (2x PE time) and doubles the input DMA bytes; bf16 keeps
the kernel at the memory roofline. Measured rel-err ~3e-3 (fro ~4e-4).
"""

import sys

if "/opt/trn_rl_repo" not in sys.path:
    sys.path.insert(0, "/opt/trn_rl_repo")

import numpy as np

B, S, E = 2, 4096, 1024
N_CORES = 8
ROWS = B * S            # 8192
R = ROWS // N_CORES     # 1024 rows per core
P = 128                 # partitions
KT = E // P             # 8 contraction tiles
MT = R // P             # 8 row tiles per core
NSZ = 512               # PSUM bank free size (fp32)
NT = E // NSZ           # 2 output-column tiles
GM = 4                  # m-tiles per output-store group
NG = MT // GM           # 2 groups

_NC_CACHE = {}


def _build_nc(**bass_kwargs):
    import concourse.bass as bass
    import concourse.mybir as mybir
    from concourse import bacc
    from concourse.tile import TileContext

    f32 = mybir.dt.float32
    bf16 = mybir.dt.bfloat16
    nc = bacc.Bacc(None, target_bir_lowering=False, **bass_kwargs)

    # xb[p, (m*KT + k)*P + mm] = x_shard[m*P + mm, k*P + p]   (bf16, host-packed)
    xb = nc.dram_tensor("xb", [P, MT * KT * P], bf16, kind="ExternalInput")
    # wv[p, k*E + j] = Wv[j, k*P + p]                          (bf16, host-packed)
    wv = nc.dram_tensor("wv", [P, KT * E], bf16, kind="ExternalInput")
    bv = nc.dram_tensor("bv", [1, E], f32, kind="ExternalInput")
    out = nc.dram_tensor("out", [R, E], f32, kind="ExternalOutput")

    with TileContext(nc) as tc:
        with (
            tc.tile_pool(name="consts", bufs=1) as consts,
            tc.tile_pool(name="xpool", bufs=1) as xpool,
            tc.tile_pool(name="opool", bufs=2) as opool,
            tc.tile_pool(name="ppool", bufs=4, space="PSUM") as ppool,
        ):
            # bias broadcast to all 128 partitions
            bias_sb = consts.tile([P, E], f32, tag="bias")
            bias_bcast = bass.AP(
                tensor=bv.tensor if hasattr(bv, "tensor") else bv,
                offset=0,
                ap=[[0, P], [1, E]],
            )
            nc.gpsimd.dma_start(out=bias_sb, in_=bias_bcast)

            # whole Wv^T in SBUF: one 2MB SWDGE DMA, 16KB/partition contiguous
            wv_sb = consts.tile([P, KT * E], bf16, tag="wv")
            nc.gpsimd.dma_start(out=wv_sb, in_=wv[:, :])

            # whole x shard in SBUF: two 1MB SWDGE DMAs (per 4-m group)
            x_sb = xpool.tile([P, MT * KT * P], bf16, tag="x")
            half = (MT // 2) * KT * P
            nc.gpsimd.dma_start(out=x_sb[:, :half], in_=xb[:, :half])
            nc.gpsimd.dma_start(out=x_sb[:, half:], in_=xb[:, half:])

            for g in range(NG):
                om = opool.tile([P, GM * E], f32, tag="om")
                for mi in range(GM):
                    m = g * GM + mi
                    for n in range(NT):
                        ps = ppool.tile([P, NSZ], f32, tag="ps")
                        for k in range(KT):
                            nc.tensor.matmul(
                                ps,
                                x_sb[:, (m * KT + k) * P : (m * KT + k + 1) * P],
                                wv_sb[:, k * E + n * NSZ : k * E + (n + 1) * NSZ],
                                start=(k == 0),
                                stop=(k == KT - 1),
                            )
                        nc.vector.tensor_add(
                            out=om[:, mi * E + n * NSZ : mi * E + (n + 1) * NSZ],
                            in0=ps,
                            in1=bias_sb[:, n * NSZ : (n + 1) * NSZ],
                        )
                # one 2MB SWDGE store for the 4-m group:
                # dst element [p, mi, j] = out[(g*GM+mi)*P + p, j]
                dst = bass.AP(
                    tensor=out.tensor if hasattr(out, "tensor") else out,
                    offset=g * GM * P * E,
                    ap=[[E, P], [P * E, GM], [1, E]],
                )
                nc.gpsimd.dma_start(
                    out=dst, in_=om.rearrange("p (mi j) -> p mi j", j=E)
                )
    nc.compile()
    return nc


def _get_nc():
    if "nc" not in _NC_CACHE:
        _NC_CACHE["nc"] = _build_nc()
    return _NC_CACHE["nc"]


def _prep_in_maps(x, Wv, bv):
    import ml_dtypes

    bf16 = ml_dtypes.bfloat16
    x = np.ascontiguousarray(np.asarray(x, dtype=np.float32))
    Wv = np.asarray(Wv, dtype=np.float32)
    bv = np.asarray(bv, dtype=np.float32)

    xf = x.reshape(ROWS, E)
    # wv[p, k*E + j] = Wv[j, k*P+p]: [j, (k p)] -> [p, (k j)]
    wvp = np.ascontiguousarray(
        Wv.reshape(E, KT, P).transpose(2, 1, 0).reshape(P, KT * E).astype(bf16)
    )
    bv2 = np.ascontiguousarray(bv.reshape(1, E))

    in_maps = []
    for c in range(N_CORES):
        xs = xf[c * R : (c + 1) * R]                    # [R, E]
        # xb[p, (m*KT+k)*P+mm] = xs[m*P+mm, k*P+p]
        xbc = np.ascontiguousarray(
            xs.reshape(MT, P, KT, P)
            .transpose(3, 0, 2, 1)
            .reshape(P, MT * KT * P)
            .astype(bf16)
        )
        in_maps.append({"xb": xbc, "wv": wvp, "bv": bv2})
    return in_maps


def _install_ntff_hook():
    """This image's antenv lacks axon_hooks; recreate the bridge module so
    run_bass_kernel_spmd(trace=True) can reach the ctypes NTFF profiler."""
    import types

    if "antenv.axon_hooks" in sys.modules:
        return
    try:
        from trn_agent_boot.trn_boot import _ntff_profile_via_ctypes
    except ImportError:
        return
    hook = _ntff_profile_via_ctypes("/opt/axon/libaxon_pjrt.so")
    mod = types.ModuleType("antenv.axon_hooks")
    mod._hook = hook
    mod.get_axon_ntff_profile_hook = lambda: mod._hook
    mod.set_axon_ntff_profile_hook = lambda h: setattr(mod, "_hook", h)
    sys.modules["antenv.axon_hooks"] = mod


def _run(x, Wv, bv, trace=False):
    from concourse.bass_utils import run_bass_kernel_spmd

    if trace:
        _install_ntff_hook()
    nc = _get_nc()
    in_maps = _prep_in_maps(x, Wv, bv)
    res = run_bass_kernel_spmd(
        nc, in_maps, core_ids=list(range(N_CORES)), trace=trace
    )
    out = np.concatenate([res.results[c]["out"] for c in range(N_CORES)], axis=0)
    return out.reshape(B, S, E).astype(np.float32), res


def kernel(x, Wq, bq, Wk, bk, Wv, bv, weights):
    out, _ = _run(x, Wv, bv, trace=False)
    return out


def kernel_traced(x, Wq, bq, Wk, bk, Wv, bv, weights):
    """Like kernel() but with NTFF profiling; returns (out, BassKernelResults)."""
    out, res = _run(x, Wv, bv, trace=True)
    return out, res


# revision 11
# speedup vs baseline: 2.5988x; 1.0987x over previous
"""Trainium2 kernel for nn_KernelizedAttention_14869176779022.

Math note: the reference computes
    out = (s * v) / s        with s = <phi_q, phi_k> > 0  (sums of exps)
so out == v == x @ Wv.T + bv exactly (up to one multiply/divide rounding).
The kernel therefore only computes the Wv linear layer.

Sharding: data-parallel over the 8192 (B*S) positions — 1024 rows per core.
Wv (pre-transposed) and bv are replicated. x is pre-swizzled on the host into
the exact SBUF layout the TensorEngine wants (contraction dim on partitions),
so every DMA is a few large per-partition-contiguous transfers via SWDGE.

Compute dtype bf16 (fp32 PSUM accumulation): fp32 matmul on TRN2 runs as a
HI/LO double pass (2x PE time) and doubles the input DMA bytes; bf16 keeps
the kernel at the memory roofline. Measured rel-err ~3e-3 (fro ~4e-4).
"""

import sys

if "/opt/trn_rl_repo" not in sys.path:
    sys.path.insert(0, "/opt/trn_rl_repo")

import numpy as np

B, S, E = 2, 4096, 1024
N_CORES = 8
ROWS = B * S            # 8192
R = ROWS // N_CORES     # 1024 rows per core
P = 128                 # partitions
KT = E // P             # 8 contraction tiles
MT = R // P             # 8 row tiles per core
NSZ = 512               # PSUM bank free size (fp32)
NT = E // NSZ           # 2 output-column tiles
GM = 2                  # m-tiles per output-store group
NG = MT // GM           # 4 groups

_NC_CACHE = {}


def _build_nc(**bass_kwargs):
    import concourse.bass as bass
    import concourse.mybir as mybir
    from concourse import bacc
    from concourse.tile import TileContext

    f32 = mybir.dt.float32
    bf16 = mybir.dt.bfloat16
    bass_kwargs.setdefault("num_swdge_queues", 4)
    nc = bacc.Bacc(None, target_bir_lowering=False, **bass_kwargs)

    # xb[p, (m*KT + k)*P + mm] = x_shard[m*P + mm, k*P + p]   (bf16, host-packed)
    xb = nc.dram_tensor("xb", [P, MT * KT * P], bf16, kind="ExternalInput")
    # wv[p, k*E + j] = Wv[j, k*P + p]                          (bf16, host-packed)
    wv = nc.dram_tensor("wv", [P, KT * E], bf16, kind="ExternalInput")
    bv = nc.dram_tensor("bv", [1, E], f32, kind="ExternalInput")
    out = nc.dram_tensor("out", [R, E], f32, kind="ExternalOutput")

    with TileContext(nc) as tc:
        with (
            tc.tile_pool(name="consts", bufs=1) as consts,
            tc.tile_pool(name="xpool", bufs=1) as xpool,
            tc.tile_pool(name="opool", bufs=2) as opool,
            tc.tile_pool(name="ppool", bufs=4, space="PSUM") as ppool,
        ):
            # bias broadcast to all 128 partitions
            bias_sb = consts.tile([P, E], f32, tag="bias")
            bias_bcast = bass.AP(
                tensor=bv.tensor if hasattr(bv, "tensor") else bv,
                offset=0,
                ap=[[0, P], [1, E]],
            )
            nc.gpsimd.dma_start(out=bias_sb, in_=bias_bcast)

            # Wv^T in SBUF, loaded as 4 chunks of 2 k-tiles (512KB each) so the
            # first matmuls unblock early; x shard as 4 chunks of 2 m-tiles.
            wv_sb = consts.tile([P, KT * E], bf16, tag="wv")
            x_sb = xpool.tile([P, MT * KT * P], bf16, tag="x")
            kc = 2 * E            # wv chunk: 2 k-tiles
            xc = 2 * KT * P       # x chunk: 2 m-tiles
            for c in range(4):
                nc.gpsimd.dma_start(
                    out=wv_sb[:, c * kc : (c + 1) * kc],
                    in_=wv[:, c * kc : (c + 1) * kc],
                )
                nc.gpsimd.dma_start(
                    out=x_sb[:, c * xc : (c + 1) * xc],
                    in_=xb[:, c * xc : (c + 1) * xc],
                )

            for g in range(NG):
                om = opool.tile([P, GM * E], f32, tag="om")
                for mi in range(GM):
                    m = g * GM + mi
                    for n in range(NT):
                        ps = ppool.tile([P, NSZ], f32, tag="ps")
                        for k in range(KT):
                            nc.tensor.matmul(
                                ps,
                                x_sb[:, (m * KT + k) * P : (m * KT + k + 1) * P],
                                wv_sb[:, k * E + n * NSZ : k * E + (n + 1) * NSZ],
                                start=(k == 0),
                                stop=(k == KT - 1),
                            )
                        nc.vector.tensor_add(
                            out=om[:, mi * E + n * NSZ : mi * E + (n + 1) * NSZ],
                            in0=ps,
                            in1=bias_sb[:, n * NSZ : (n + 1) * NSZ],
                        )
                # one SWDGE store per GM-m group:
                # dst element [p, mi, j] = out[(g*GM+mi)*P + p, j]
                dst = bass.AP(
                    tensor=out.tensor if hasattr(out, "tensor") else out,
                    offset=g * GM * P * E,
                    ap=[[E, P], [P * E, GM], [1, E]],
                )
                nc.gpsimd.dma_start(
                    out=dst, in_=om.rearrange("p (mi j) -> p mi j", j=E)
                )
    nc.compile()
    return nc


def _get_nc():
    if "nc" not in _NC_CACHE:
        _NC_CACHE["nc"] = _build_nc()
    return _NC_CACHE["nc"]


def _prep_in_maps(x, Wv, bv):
    import ml_dtypes

    bf16 = ml_dtypes.bfloat16
    x = np.ascontiguousarray(np.asarray(x, dtype=np.float32))
    Wv = np.asarray(Wv, dtype=np.float32)
    bv = np.asarray(bv, dtype=np.float32)

    xf = x.reshape(ROWS, E)
    # wv[p, k*E + j] = Wv[j, k*P+p]: [j, (k p)] -> [p, (k j)]
    wvp = np.ascontiguousarray(
        Wv.reshape(E, KT, P).transpose(2, 1, 0).reshape(P, KT * E).astype(bf16)
    )
    bv2 = np.ascontiguousarray(bv.reshape(1, E))

    in_maps = []
    for c in range(N_CORES):
        xs = xf[c * R : (c + 1) * R]                    # [R, E]
        # xb[p, (m*KT+k)*P+mm] = xs[m*P+mm, k*P+p]
        xbc = np.ascontiguousarray(
            xs.reshape(MT, P, KT, P)
            .transpose(3, 0, 2, 1)
            .reshape(P, MT * KT * P)
            .astype(bf16)
        )
        in_maps.append({"xb": xbc, "wv": wvp, "bv": bv2})
    return in_maps


def _install_ntff_hook():
    """This image's antenv lacks axon_hooks; recreate the bridge module so
    run_bass_kernel_spmd(trace=True) can reach the ctypes NTFF profiler."""
    import types

    if "antenv.axon_hooks" in sys.modules:
        return
    try:
        from trn_agent_boot.trn_boot import _ntff_profile_via_ctypes
    except ImportError:
        return
    hook = _ntff_profile_via_ctypes("/opt/axon/libaxon_pjrt.so")
    mod = types.ModuleType("antenv.axon_hooks")
    mod._hook = hook
    mod.get_axon_ntff_profile_hook = lambda: mod._hook
    mod.set_axon_ntff_profile_hook = lambda h: setattr(mod, "_hook", h)
    sys.modules["antenv.axon_hooks"] = mod


def _run(x, Wv, bv, trace=False):
    from concourse.bass_utils import run_bass_kernel_spmd

    if trace:
        _install_ntff_hook()
    nc = _get_nc()
    in_maps = _prep_in_maps(x, Wv, bv)
    res = run_bass_kernel_spmd(
        nc, in_maps, core_ids=list(range(N_CORES)), trace=trace
    )
    out = np.concatenate([res.results[c]["out"] for c in range(N_CORES)], axis=0)
    return out.reshape(B, S, E).astype(np.float32), res


def kernel(x, Wq, bq, Wk, bk, Wv, bv, weights):
    out, _ = _run(x, Wv, bv, trace=False)
    return out


def kernel_traced(x, Wq, bq, Wk, bk, Wv, bv, weights):
    """Like kernel() but with NTFF profiling; returns (out, BassKernelResults)."""
    out, res = _run(x, Wv, bv, trace=True)
    return out, res
